# revision 8
# baseline (speedup 1.0000x reference)
"""Trainium2 Bass kernel for the ternary-MLP decoder.

  h   = tanh(x @ (s1 * tern(w1 - scale*n1)) + b1)
  out = (h @ (s2 * tern(w2 - scale*n2)) + b2).reshape(-1, 3, 32, 32)

Strategy (8 NeuronCores, Megatron tensor-parallel over D_H):
  - core c owns h-columns [c*2048, (c+1)*2048): w1/s1/b1 column shard,
    w2 row shard. Full batch on every core.
  - All matmuls computed in transposed space: hT = t1c.T @ xT,
    poutT = t2c.T @ hT, so the natural [K, M] layouts of w1/w2 feed the
    PE stationary operand directly and no on-device transposes happen.
    Host passes xT (bf16) and reassembles outT at the end.
  - Ternarization runs on-device in exact fp32 (bitwise-identical
    compares to the reference); ternary weights are cast to bf16 which
    is exact for {-1, 0, 1}. Matmuls run bf16 with fp32 PSUM accum.
  - The [3072, B] partial outputs are summed across cores with a
    chunked ReduceScatter (batch chunks of 512) that overlaps compute;
    core c ends up with outT rows [c*384, (c+1)*384) for all batch.
"""

import os
from contextlib import ExitStack

import ml_dtypes
import numpy as np

import concourse.bass as bass
import concourse.tile as tile
from concourse import bacc, mybir
from concourse.bass_utils import run_bass_kernel_spmd

F32 = mybir.dt.float32
BF16 = mybir.dt.bfloat16

# Problem dims (hardcoded per contract).
B, DIN, DH, DOUT = 4096, 1024, 16384, 3 * 32 * 32
W = 8  # cores

# Results of the last traced run (for test harness inspection).
LAST_RUN = None


def build_decoder_nc(
    scale: float,
    b: int = B,
    din: int = DIN,
    dh: int = DH,
    dout: int = DOUT,
    w: int = W,
    cb: int = 512,
    tern_f: int = 512,
):
    """Build the per-core Bass program (same program for all cores; the
    per-core shards arrive as inputs)."""
    P = 128
    hsh = dh // w        # h columns owned by this core
    osh = dout // w      # outT rows owned after ReduceScatter
    nkt1 = din // P      # L1 contraction tiles
    nkt2 = hsh // P      # L2 contraction tiles (== L1 output m-tiles)
    nm1 = hsh // P       # L1 output tiles (hT rows / P)
    nm2 = dout // P      # L2 output tiles (outT rows / P)
    nor = osh // P       # post-RS row tiles
    nch = b // cb        # batch chunks
    assert din % P == 0 and hsh % P == 0 and dout % P == 0 and osh % P == 0
    assert b % cb == 0 and cb <= 512

    nc = bacc.Bacc(None, num_devices=w)

    xT = nc.dram_tensor("xT", [din, b], BF16, kind="ExternalInput")
    w1c = nc.dram_tensor("w1c", [din, hsh], F32, kind="ExternalInput")
    n1c = nc.dram_tensor("n1c", [din, hsh], F32, kind="ExternalInput")
    w2c = nc.dram_tensor("w2c", [hsh, dout], F32, kind="ExternalInput")
    n2c = nc.dram_tensor("n2c", [hsh, dout], F32, kind="ExternalInput")
    s1c = nc.dram_tensor("s1c", [P, nm1], F32, kind="ExternalInput")
    b1c = nc.dram_tensor("b1c", [P, nm1], F32, kind="ExternalInput")
    s2c = nc.dram_tensor("s2c", [P, nor], F32, kind="ExternalInput")
    b2c = nc.dram_tensor("b2c", [P, nor], F32, kind="ExternalInput")
    outT = nc.dram_tensor("outT", [osh, b], F32, kind="ExternalOutput")

    # Per-chunk DRAM buffers for the cross-core reduction.
    partials = [nc.dram_tensor(f"partial_{i}", [dout, cb], F32) for i in range(nch)]
    rs_outs = [
        nc.dram_tensor(f"rs_out_{i}", [osh, cb], F32) for i in range(nch)
    ]
    groups = [list(range(w))]

    xT3 = xT.rearrange("(ko p) b -> p ko b", p=P)

    with TileCtx(nc) as tc, ExitStack() as ctx:
        consts = ctx.enter_context(tc.tile_pool(name="consts", bufs=1))
        t1p = ctx.enter_context(tc.tile_pool(name="t1", bufs=1))
        t2p = ctx.enter_context(tc.tile_pool(name="t2", bufs=1))
        wnp = ctx.enter_context(tc.tile_pool(name="wn", bufs=3))
        gtp = ctx.enter_context(tc.tile_pool(name="gt", bufs=2))
        xp = ctx.enter_context(tc.tile_pool(name="xb", bufs=2))
        hp = ctx.enter_context(tc.tile_pool(name="h", bufs=2))
        stp = ctx.enter_context(tc.tile_pool(name="stage", bufs=3))
        rsp = ctx.enter_context(tc.tile_pool(name="rsp", bufs=2))
        ps1 = ctx.enter_context(tc.tile_pool(name="ps1", bufs=2, space="PSUM"))
        ps2 = ctx.enter_context(tc.tile_pool(name="ps2", bufs=4, space="PSUM"))

        # Per-partition scale/bias vectors (host pre-arranged as [128, m]).
        s1_sb = consts.tile([P, nm1], F32, tag="s1")
        b1_sb = consts.tile([P, nm1], F32, tag="b1")
        s2_sb = consts.tile([P, nor], F32, tag="s2")
        b2_sb = consts.tile([P, nor], F32, tag="b2")
        nc.sync.dma_start(s1_sb[:], s1c[:])
        nc.sync.dma_start(b1_sb[:], b1c[:])
        nc.sync.dma_start(s2_sb[:], s2c[:])
        nc.sync.dma_start(b2_sb[:], b2c[:])

        def ternarize_ktile(dst, w_dram, n_dram, kt, fdim):
            """dst[:, 0:fdim] (bf16) = (q > 1) - (q < -1), q = w - scale*n.

            Exact fp32 compares, identical to the reference semantics."""
            fw = min(tern_f, fdim)
            assert fdim % fw == 0
            for j in range(fdim // fw):
                cols = slice(j * fw, (j + 1) * fw)
                wt = wnp.tile([P, fw], F32, tag="w")
                nc.sync.dma_start(wt[:], w_dram[kt * P : (kt + 1) * P, cols])
                if scale != 0.0:
                    nt = wnp.tile([P, fw], F32, tag="n")
                    nc.sync.dma_start(nt[:], n_dram[kt * P : (kt + 1) * P, cols])
                    if scale != 1.0:
                        nc.vector.tensor_scalar(
                            nt[:], nt[:], float(scale), None, mybir.AluOpType.mult
                        )
                    nc.vector.tensor_tensor(
                        wt[:], wt[:], nt[:], mybir.AluOpType.subtract
                    )
                gt = gtp.tile([P, fw], F32, tag="gt")
                nc.vector.tensor_scalar(
                    gt[:], wt[:], 1.0, None, mybir.AluOpType.is_gt
                )
                nc.vector.tensor_scalar(
                    wt[:], wt[:], -1.0, None, mybir.AluOpType.is_lt
                )
                nc.vector.tensor_tensor(
                    dst[:, cols], gt[:], wt[:], mybir.AluOpType.subtract
                )

        # Resident ternary weights (bf16): t1 = 8 x [128, hsh], t2 = 16 x [128, dout].
        t1_sb = [
            t1p.tile([P, hsh], BF16, tag=f"t1_{k}", name=f"t1_{k}")
            for k in range(nkt1)
        ]
        for kt in range(nkt1):
            ternarize_ktile(t1_sb[kt], w1c, n1c, kt, hsh)
        t2_sb = [
            t2p.tile([P, dout], BF16, tag=f"t2_{k}", name=f"t2_{k}")
            for k in range(nkt2)
        ]
        for kt in range(nkt2):
            ternarize_ktile(t2_sb[kt], w2c, n2c, kt, dout)

        for ch in range(nch):
            bcols = slice(ch * cb, (ch + 1) * cb)

            # Batch chunk of xT: [128, nkt1, cb] bf16.
            xb = xp.tile([P, nkt1, cb], BF16, tag="xb")
            nc.sync.dma_start(xb[:], xT3[:, :, bcols])

            # L1: hT[m] = tanh((t1.T @ xT)[m] * s1[m] + b1[m])  (bf16)
            h = hp.tile([P, nm1, cb], BF16, tag="h")
            for m in range(nm1):
                acc = ps1.tile([P, cb], F32, tag="ps1")
                for kt in range(nkt1):
                    nc.tensor.matmul(
                        acc[:],
                        t1_sb[kt][:, m * P : (m + 1) * P],
                        xb[:, kt, :],
                        start=(kt == 0),
                        stop=(kt == nkt1 - 1),
                    )
                nc.scalar.activation(
                    h[:, m, :],
                    acc[:],
                    mybir.ActivationFunctionType.Tanh,
                    bias=b1_sb[:, m : m + 1],
                    scale=s1_sb[:, m : m + 1],
                )

            # L2: poutT[mo] = (t2.T @ hT)[mo]  (fp32 partial, pre s2/b2)
            for mo in range(nm2):
                acc = ps2.tile([P, cb], F32, tag="ps2")
                for kt in range(nkt2):
                    nc.tensor.matmul(
                        acc[:],
                        t2_sb[kt][:, mo * P : (mo + 1) * P],
                        h[:, kt, :],
                        start=(kt == 0),
                        stop=(kt == nkt2 - 1),
                    )
                st = stp.tile([P, cb], F32, tag="st")
                nc.vector.tensor_copy(st[:], acc[:])
                nc.sync.dma_start(
                    partials[ch][mo * P : (mo + 1) * P, :], st[:]
                )

            # Sum partials across cores; core c keeps outT rows [c*osh, (c+1)*osh).
            nc.gpsimd.collective_compute(
                "ReduceScatter",
                mybir.AluOpType.add,
                replica_groups=groups,
                ins=[partials[ch][:]],
                outs=[rs_outs[ch][:]],
            )

            # Apply s2/b2 to the owned slab and emit.
            for r in range(nor):
                rt = rsp.tile([P, cb], F32, tag="rs")
                nc.sync.dma_start(rt[:], rs_outs[ch][r * P : (r + 1) * P, :])
                nc.vector.tensor_scalar(
                    rt[:],
                    rt[:],
                    s2_sb[:, r : r + 1],
                    b2_sb[:, r : r + 1],
                    mybir.AluOpType.mult,
                    mybir.AluOpType.add,
                )
                nc.sync.dma_start(outT[r * P : (r + 1) * P, bcols], rt[:])

    nc.compile()
    return nc


def TileCtx(nc):
    return tile.TileContext(nc)


def _shard_inputs(x, w1, s1, b1, w2, s2, b2, n1, n2, w=W, dh=DH, dout=DOUT):
    P = 128
    hsh = dh // w
    osh = dout // w
    nm1 = hsh // P
    nor = osh // P
    xT = np.ascontiguousarray(x.T).astype(ml_dtypes.bfloat16)
    in_maps = []
    for c in range(w):
        hs = slice(c * hsh, (c + 1) * hsh)
        os_ = slice(c * osh, (c + 1) * osh)
        in_maps.append(
            {
                "xT": xT,
                "w1c": np.ascontiguousarray(w1[:, hs]),
                "n1c": np.ascontiguousarray(n1[:, hs]),
                "w2c": np.ascontiguousarray(w2[hs, :]),
                "n2c": np.ascontiguousarray(n2[hs, :]),
                "s1c": np.ascontiguousarray(s1[hs].reshape(nm1, P).T),
                "b1c": np.ascontiguousarray(b1[hs].reshape(nm1, P).T),
                "s2c": np.ascontiguousarray(s2[os_].reshape(nor, P).T),
                "b2c": np.ascontiguousarray(b2[os_].reshape(nor, P).T),
            }
        )
    return in_maps


_NC_CACHE = {}


def kernel(**inputs) -> np.ndarray:
    global LAST_RUN
    x = np.asarray(inputs["x"], dtype=np.float32)
    w1 = np.asarray(inputs["w1"], dtype=np.float32)
    s1 = np.asarray(inputs["s1"], dtype=np.float32)
    b1 = np.asarray(inputs["b1"], dtype=np.float32)
    w2 = np.asarray(inputs["w2"], dtype=np.float32)
    s2 = np.asarray(inputs["s2"], dtype=np.float32)
    b2 = np.asarray(inputs["b2"], dtype=np.float32)
    n1 = np.asarray(inputs["n1"], dtype=np.float32)
    n2 = np.asarray(inputs["n2"], dtype=np.float32)
    scale = float(np.asarray(inputs["scale"]))

    key = scale
    if key not in _NC_CACHE:
        _NC_CACHE[key] = build_decoder_nc(scale)
    nc = _NC_CACHE[key]

    in_maps = _shard_inputs(x, w1, s1, b1, w2, s2, b2, n1, n2)
    trace = bool(int(os.environ.get("KERNEL_TRACE", "0")))
    res = run_bass_kernel_spmd(
        nc, in_maps, core_ids=list(range(W)), trace=trace
    )
    LAST_RUN = res

    osh = DOUT // W
    outT = np.concatenate(
        [np.asarray(res.results[c]["outT"]) for c in range(W)], axis=0
    )
    assert outT.shape == (DOUT, B)
    out = np.ascontiguousarray(outT.T).reshape(B, 3, 32, 32).astype(np.float32)
    return out


# revision 9
# speedup vs baseline: 1.0926x; 1.0926x over previous
"""Trainium2 Bass kernel for the ternary-MLP decoder.

  h   = tanh(x @ (s1 * tern(w1 - scale*n1)) + b1)
  out = (h @ (s2 * tern(w2 - scale*n2)) + b2).reshape(-1, 3, 32, 32)

Strategy (8 NeuronCores, Megatron tensor-parallel over D_H):
  - core c owns h-columns [c*2048, (c+1)*2048): w1/s1/b1 column shard,
    w2 row shard. Full batch on every core.
  - All matmuls computed in transposed space: hT = t1c.T @ xT,
    poutT = t2c.T @ hT, so the natural [K, M] layouts of w1/w2 feed the
    PE stationary operand directly and no on-device transposes happen.
    Host passes xT (bf16) and reassembles outT at the end.
  - Ternarization runs on-device in exact fp32 (bitwise-identical
    compares to the reference); ternary weights are cast to bf16 which
    is exact for {-1, 0, 1}. Matmuls run bf16 with fp32 PSUM accum.
  - The [3072, B] partial outputs are summed across cores with a
    chunked ReduceScatter (batch chunks of 512) that overlaps compute;
    core c ends up with outT rows [c*384, (c+1)*384) for all batch.
"""

import os
from contextlib import ExitStack

import ml_dtypes
import numpy as np

import concourse.bass as bass
import concourse.tile as tile
from concourse import bacc, mybir
from concourse.bass_utils import run_bass_kernel_spmd

F32 = mybir.dt.float32
BF16 = mybir.dt.bfloat16
FP8 = mybir.dt.float8e4

# Problem dims (hardcoded per contract).
B, DIN, DH, DOUT = 4096, 1024, 16384, 3 * 32 * 32
W = 8  # cores

# Results of the last traced run (for test harness inspection).
LAST_RUN = None


def build_decoder_nc(
    scale: float,
    b: int = B,
    din: int = DIN,
    dh: int = DH,
    dout: int = DOUT,
    w: int = W,
    cb: int = 512,
    tern_f: int = 512,
):
    """Build the per-core Bass program (same program for all cores; the
    per-core shards arrive as inputs)."""
    P = 128
    hsh = dh // w        # h columns owned by this core
    osh = dout // w      # outT rows owned after ReduceScatter
    nkt1 = din // P      # L1 contraction tiles
    nkt2 = hsh // P      # L2 contraction tiles (== L1 output m-tiles)
    nm1 = hsh // P       # L1 output tiles (hT rows / P)
    nm2 = dout // P      # L2 output tiles (outT rows / P)
    nor = osh // P       # post-RS row tiles
    nch = b // cb        # batch chunks
    assert din % P == 0 and hsh % P == 0 and dout % P == 0 and osh % P == 0
    assert b % cb == 0 and cb <= 512

    nc = bacc.Bacc(None, num_devices=w)

    xT = nc.dram_tensor("xT", [din, b], BF16, kind="ExternalInput")
    w1c = nc.dram_tensor("w1c", [din, hsh], F32, kind="ExternalInput")
    n1c = nc.dram_tensor("n1c", [din, hsh], F32, kind="ExternalInput")
    w2c = nc.dram_tensor("w2c", [hsh, dout], F32, kind="ExternalInput")
    n2c = nc.dram_tensor("n2c", [hsh, dout], F32, kind="ExternalInput")
    s1c = nc.dram_tensor("s1c", [P, nm1], F32, kind="ExternalInput")
    b1c = nc.dram_tensor("b1c", [P, nm1], F32, kind="ExternalInput")
    s2c = nc.dram_tensor("s2c", [P, nor], F32, kind="ExternalInput")
    b2c = nc.dram_tensor("b2c", [P, nor], F32, kind="ExternalInput")
    outT = nc.dram_tensor("outT", [osh, b], F32, kind="ExternalOutput")

    # Per-chunk DRAM buffers for the cross-core reduction.
    partials = [nc.dram_tensor(f"partial_{i}", [dout, cb], F32) for i in range(nch)]
    rs_outs = [
        nc.dram_tensor(f"rs_out_{i}", [osh, cb], F32) for i in range(nch)
    ]
    groups = [list(range(w))]

    xT3 = xT.rearrange("(ko p) b -> p ko b", p=P)

    with TileCtx(nc) as tc, ExitStack() as ctx:
        consts = ctx.enter_context(tc.tile_pool(name="consts", bufs=1))
        t1p = ctx.enter_context(tc.tile_pool(name="t1", bufs=1))
        t2p = ctx.enter_context(tc.tile_pool(name="t2", bufs=1))
        wnp = ctx.enter_context(tc.tile_pool(name="wn", bufs=3))
        gtp = ctx.enter_context(tc.tile_pool(name="gt", bufs=2))
        xp = ctx.enter_context(tc.tile_pool(name="xb", bufs=4))
        hp = ctx.enter_context(tc.tile_pool(name="h", bufs=4))
        stp = ctx.enter_context(tc.tile_pool(name="stage", bufs=8))
        rsp = ctx.enter_context(tc.tile_pool(name="rsp", bufs=4))
        ps1 = ctx.enter_context(tc.tile_pool(name="ps1", bufs=2, space="PSUM"))
        ps2 = ctx.enter_context(tc.tile_pool(name="ps2", bufs=6, space="PSUM"))

        # Per-partition scale/bias vectors (host pre-arranged as [128, m]).
        s1_sb = consts.tile([P, nm1], F32, tag="s1")
        b1_sb = consts.tile([P, nm1], F32, tag="b1")
        s2_sb = consts.tile([P, nor], F32, tag="s2")
        b2_sb = consts.tile([P, nor], F32, tag="b2")
        nc.sync.dma_start(s1_sb[:], s1c[:])
        nc.sync.dma_start(b1_sb[:], b1c[:])
        nc.sync.dma_start(s2_sb[:], s2c[:])
        nc.sync.dma_start(b2_sb[:], b2c[:])

        def ternarize_ktile(dst, w_dram, n_dram, kt, fdim):
            """dst[:, 0:fdim] (bf16) = (q > 1) - (q < -1), q = w - scale*n.

            Exact fp32 compares, identical to the reference semantics."""
            fw = min(tern_f, fdim)
            assert fdim % fw == 0
            for j in range(fdim // fw):
                cols = slice(j * fw, (j + 1) * fw)
                wt = wnp.tile([P, fw], F32, tag="w")
                nc.sync.dma_start(wt[:], w_dram[kt * P : (kt + 1) * P, cols])
                if scale != 0.0:
                    nt = wnp.tile([P, fw], F32, tag="n")
                    nc.sync.dma_start(nt[:], n_dram[kt * P : (kt + 1) * P, cols])
                    if scale != 1.0:
                        nc.vector.tensor_scalar(
                            nt[:], nt[:], float(scale), None, mybir.AluOpType.mult
                        )
                    nc.vector.tensor_tensor(
                        wt[:], wt[:], nt[:], mybir.AluOpType.subtract
                    )
                gt = gtp.tile([P, fw], F32, tag="gt")
                nc.vector.tensor_scalar(
                    gt[:], wt[:], 1.0, None, mybir.AluOpType.is_gt
                )
                nc.vector.tensor_scalar(
                    wt[:], wt[:], -1.0, None, mybir.AluOpType.is_lt
                )
                nc.vector.tensor_tensor(
                    dst[:, cols], gt[:], wt[:], mybir.AluOpType.subtract
                )

        # Resident ternary weights (bf16): t1 = 8 x [128, hsh], t2 = 16 x [128, dout].
        t1_sb = [
            t1p.tile([P, hsh], FP8, tag=f"t1_{k}", name=f"t1_{k}")
            for k in range(nkt1)
        ]
        for kt in range(nkt1):
            ternarize_ktile(t1_sb[kt], w1c, n1c, kt, hsh)
        t2_sb = [
            t2p.tile([P, dout], FP8, tag=f"t2_{k}", name=f"t2_{k}")
            for k in range(nkt2)
        ]
        for kt in range(nkt2):
            ternarize_ktile(t2_sb[kt], w2c, n2c, kt, dout)

        for ch in range(nch):
            bcols = slice(ch * cb, (ch + 1) * cb)

            # Batch chunk of xT: [128, nkt1, cb] bf16.
            xb = xp.tile([P, nkt1, cb], BF16, tag="xb")
            nc.sync.dma_start(xb[:], xT3[:, :, bcols])

            # L1: hT[m] = tanh((t1.T @ xT)[m] * s1[m] + b1[m])  (bf16)
            h = hp.tile([P, nm1, cb], BF16, tag="h")
            for m in range(nm1):
                acc = ps1.tile([P, cb], F32, tag="ps1")
                for kt in range(nkt1):
                    nc.tensor.matmul(
                        acc[:],
                        t1_sb[kt][:, m * P : (m + 1) * P],
                        xb[:, kt, :],
                        start=(kt == 0),
                        stop=(kt == nkt1 - 1),
                    )
                nc.scalar.activation(
                    h[:, m, :],
                    acc[:],
                    mybir.ActivationFunctionType.Tanh,
                    bias=b1_sb[:, m : m + 1],
                    scale=s1_sb[:, m : m + 1],
                )

            # L2: poutT[mo] = (t2.T @ hT)[mo]  (fp32 partial, pre s2/b2)
            for mo in range(nm2):
                acc = ps2.tile([P, cb], F32, tag="ps2")
                for kt in range(nkt2):
                    nc.tensor.matmul(
                        acc[:],
                        t2_sb[kt][:, mo * P : (mo + 1) * P],
                        h[:, kt, :],
                        start=(kt == 0),
                        stop=(kt == nkt2 - 1),
                    )
                st = stp.tile([P, cb], F32, tag="st")
                nc.vector.tensor_copy(st[:], acc[:])
                nc.sync.dma_start(
                    partials[ch][mo * P : (mo + 1) * P, :], st[:]
                )

            # Sum partials across cores; core c keeps outT rows [c*osh, (c+1)*osh).
            nc.gpsimd.collective_compute(
                "ReduceScatter",
                mybir.AluOpType.add,
                replica_groups=groups,
                ins=[partials[ch][:]],
                outs=[rs_outs[ch][:]],
            )

            # Apply s2/b2 to the owned slab and emit.
            for r in range(nor):
                rt = rsp.tile([P, cb], F32, tag="rs")
                nc.sync.dma_start(rt[:], rs_outs[ch][r * P : (r + 1) * P, :])
                nc.vector.tensor_scalar(
                    rt[:],
                    rt[:],
                    s2_sb[:, r : r + 1],
                    b2_sb[:, r : r + 1],
                    mybir.AluOpType.mult,
                    mybir.AluOpType.add,
                )
                nc.sync.dma_start(outT[r * P : (r + 1) * P, bcols], rt[:])

    nc.compile()
    return nc


def TileCtx(nc):
    return tile.TileContext(nc)


def _shard_inputs(x, w1, s1, b1, w2, s2, b2, n1, n2, w=W, dh=DH, dout=DOUT):
    P = 128
    hsh = dh // w
    osh = dout // w
    nm1 = hsh // P
    nor = osh // P
    xT = np.ascontiguousarray(x.T).astype(ml_dtypes.bfloat16)
    in_maps = []
    for c in range(w):
        hs = slice(c * hsh, (c + 1) * hsh)
        os_ = slice(c * osh, (c + 1) * osh)
        in_maps.append(
            {
                "xT": xT,
                "w1c": np.ascontiguousarray(w1[:, hs]),
                "n1c": np.ascontiguousarray(n1[:, hs]),
                "w2c": np.ascontiguousarray(w2[hs, :]),
                "n2c": np.ascontiguousarray(n2[hs, :]),
                "s1c": np.ascontiguousarray(s1[hs].reshape(nm1, P).T),
                "b1c": np.ascontiguousarray(b1[hs].reshape(nm1, P).T),
                "s2c": np.ascontiguousarray(s2[os_].reshape(nor, P).T),
                "b2c": np.ascontiguousarray(b2[os_].reshape(nor, P).T),
            }
        )
    return in_maps


_NC_CACHE = {}


def kernel(**inputs) -> np.ndarray:
    global LAST_RUN
    x = np.asarray(inputs["x"], dtype=np.float32)
    w1 = np.asarray(inputs["w1"], dtype=np.float32)
    s1 = np.asarray(inputs["s1"], dtype=np.float32)
    b1 = np.asarray(inputs["b1"], dtype=np.float32)
    w2 = np.asarray(inputs["w2"], dtype=np.float32)
    s2 = np.asarray(inputs["s2"], dtype=np.float32)
    b2 = np.asarray(inputs["b2"], dtype=np.float32)
    n1 = np.asarray(inputs["n1"], dtype=np.float32)
    n2 = np.asarray(inputs["n2"], dtype=np.float32)
    scale = float(np.asarray(inputs["scale"]))

    key = scale
    if key not in _NC_CACHE:
        _NC_CACHE[key] = build_decoder_nc(scale)
    nc = _NC_CACHE[key]

    in_maps = _shard_inputs(x, w1, s1, b1, w2, s2, b2, n1, n2)
    trace = bool(int(os.environ.get("KERNEL_TRACE", "0")))
    res = run_bass_kernel_spmd(
        nc, in_maps, core_ids=list(range(W)), trace=trace
    )
    LAST_RUN = res

    osh = DOUT // W
    outT = np.concatenate(
        [np.asarray(res.results[c]["outT"]) for c in range(W)], axis=0
    )
    assert outT.shape == (DOUT, B)
    out = np.ascontiguousarray(outT.T).reshape(B, 3, 32, 32).astype(np.float32)
    return out


# revision 10
# speedup vs baseline: 1.2034x; 1.1014x over previous
"""Trainium2 Bass kernel for the ternary-MLP decoder.

  h   = tanh(x @ (s1 * tern(w1 - scale*n1)) + b1)
  out = (h @ (s2 * tern(w2 - scale*n2)) + b2).reshape(-1, 3, 32, 32)

Strategy (8 NeuronCores, Megatron tensor-parallel over D_H):
  - core c owns h-columns [c*2048, (c+1)*2048): w1/s1/b1 column shard,
    w2 row shard. Full batch on every core.
  - All matmuls computed in transposed space: hT = t1c.T @ xT,
    poutT = t2c.T @ hT, so the natural [K, M] layouts of w1/w2 feed the
    PE stationary operand directly and no on-device transposes happen.
    Host passes xT (bf16) and reassembles outT at the end.
  - Ternarization runs on-device in exact fp32 (bitwise-identical
    compares to the reference); ternary weights are cast to bf16 which
    is exact for {-1, 0, 1}. Matmuls run bf16 with fp32 PSUM accum.
  - The [3072, B] partial outputs are summed across cores with a
    chunked ReduceScatter (batch chunks of 512) that overlaps compute;
    core c ends up with outT rows [c*384, (c+1)*384) for all batch.
"""

import os
from contextlib import ExitStack

import ml_dtypes
import numpy as np

import concourse.bass as bass
import concourse.tile as tile
from concourse import bacc, mybir
from concourse.bass_utils import run_bass_kernel_spmd

F32 = mybir.dt.float32
BF16 = mybir.dt.bfloat16
FP8 = mybir.dt.float8e4

# Problem dims (hardcoded per contract).
B, DIN, DH, DOUT = 4096, 1024, 16384, 3 * 32 * 32
W = 8  # cores

# Results of the last traced run (for test harness inspection).
LAST_RUN = None


def build_decoder_nc(
    scale: float,
    b: int = B,
    din: int = DIN,
    dh: int = DH,
    dout: int = DOUT,
    w: int = W,
    cb: int = 512,
    tern_f: int = 512,
):
    """Build the per-core Bass program (same program for all cores; the
    per-core shards arrive as inputs)."""
    P = 128
    hsh = dh // w        # h columns owned by this core
    osh = dout // w      # outT rows owned after ReduceScatter
    nkt1 = din // P      # L1 contraction tiles
    nkt2 = hsh // P      # L2 contraction tiles (== L1 output m-tiles)
    nm1 = hsh // P       # L1 output tiles (hT rows / P)
    nm2 = dout // P      # L2 output tiles (outT rows / P)
    nor = osh // P       # post-RS row tiles
    nch = b // cb        # batch chunks
    assert din % P == 0 and hsh % P == 0 and dout % P == 0 and osh % P == 0
    assert b % cb == 0 and cb <= 512

    nc = bacc.Bacc(None, num_devices=w)

    xT = nc.dram_tensor("xT", [din, b], BF16, kind="ExternalInput")
    w1c = nc.dram_tensor("w1c", [din, hsh], F32, kind="ExternalInput")
    n1c = nc.dram_tensor("n1c", [din, hsh], F32, kind="ExternalInput")
    w2c = nc.dram_tensor("w2c", [hsh, dout], F32, kind="ExternalInput")
    n2c = nc.dram_tensor("n2c", [hsh, dout], F32, kind="ExternalInput")
    s1c = nc.dram_tensor("s1c", [P, nm1], F32, kind="ExternalInput")
    b1c = nc.dram_tensor("b1c", [P, nm1], F32, kind="ExternalInput")
    s2c = nc.dram_tensor("s2c", [P, nor], F32, kind="ExternalInput")
    b2c = nc.dram_tensor("b2c", [P, nor], F32, kind="ExternalInput")
    outT = nc.dram_tensor("outT", [osh, b], F32, kind="ExternalOutput")

    # Per-chunk DRAM buffers for the cross-core reduction. Each chunk's
    # [dout, cb] partial is reduced as `nrs` independent ReduceScatters of
    # w*P rows each, so every RS yields exactly one [P, cb] tile per core
    # and the HBM bursts stay small enough to hide behind compute.
    nrs = nor  # one RS group per post-RS row tile
    rs_rows = dout // nrs
    assert rs_rows % (w * P) == 0 and rs_rows // w == P
    partials = [
        [nc.dram_tensor(f"partial_{i}_{g}", [rs_rows, cb], F32) for g in range(nrs)]
        for i in range(nch)
    ]
    rs_outs = [
        [nc.dram_tensor(f"rs_out_{i}_{g}", [P, cb], F32) for g in range(nrs)]
        for i in range(nch)
    ]
    groups = [list(range(w))]
    mo_per_g = nm2 // nrs

    xT3 = xT.rearrange("(ko p) b -> p ko b", p=P)

    with TileCtx(nc) as tc, ExitStack() as ctx:
        consts = ctx.enter_context(tc.tile_pool(name="consts", bufs=1))
        t1p = ctx.enter_context(tc.tile_pool(name="t1", bufs=1))
        t2p = ctx.enter_context(tc.tile_pool(name="t2", bufs=1))
        wnp = ctx.enter_context(tc.tile_pool(name="wn", bufs=3))
        gtp = ctx.enter_context(tc.tile_pool(name="gt", bufs=2))
        xp = ctx.enter_context(tc.tile_pool(name="xb", bufs=4))
        hp = ctx.enter_context(tc.tile_pool(name="h", bufs=4))
        stp = ctx.enter_context(tc.tile_pool(name="stage", bufs=8))
        rsp = ctx.enter_context(tc.tile_pool(name="rsp", bufs=4))
        ps1 = ctx.enter_context(tc.tile_pool(name="ps1", bufs=2, space="PSUM"))
        ps2 = ctx.enter_context(tc.tile_pool(name="ps2", bufs=6, space="PSUM"))

        # Per-partition scale/bias vectors (host pre-arranged as [128, m]).
        s1_sb = consts.tile([P, nm1], F32, tag="s1")
        b1_sb = consts.tile([P, nm1], F32, tag="b1")
        s2_sb = consts.tile([P, nor], F32, tag="s2")
        b2_sb = consts.tile([P, nor], F32, tag="b2")
        nc.sync.dma_start(s1_sb[:], s1c[:])
        nc.sync.dma_start(b1_sb[:], b1c[:])
        nc.sync.dma_start(s2_sb[:], s2c[:])
        nc.sync.dma_start(b2_sb[:], b2c[:])

        def ternarize_ktile(dst, w_dram, n_dram, kt, fdim):
            """dst[:, 0:fdim] (bf16) = (q > 1) - (q < -1), q = w - scale*n.

            Exact fp32 compares, identical to the reference semantics."""
            fw = min(tern_f, fdim)
            assert fdim % fw == 0
            for j in range(fdim // fw):
                cols = slice(j * fw, (j + 1) * fw)
                wt = wnp.tile([P, fw], F32, tag="w")
                nc.sync.dma_start(wt[:], w_dram[kt * P : (kt + 1) * P, cols])
                if scale != 0.0:
                    nt = wnp.tile([P, fw], F32, tag="n")
                    nc.sync.dma_start(nt[:], n_dram[kt * P : (kt + 1) * P, cols])
                    if scale != 1.0:
                        nc.vector.tensor_scalar(
                            nt[:], nt[:], float(scale), None, mybir.AluOpType.mult
                        )
                    nc.vector.tensor_tensor(
                        wt[:], wt[:], nt[:], mybir.AluOpType.subtract
                    )
                gt = gtp.tile([P, fw], BF16, tag="gt")
                nc.vector.tensor_scalar(
                    gt[:], wt[:], 1.0, None, mybir.AluOpType.is_gt
                )
                lt = gtp.tile([P, fw], BF16, tag="lt")
                nc.vector.tensor_scalar(
                    lt[:], wt[:], -1.0, None, mybir.AluOpType.is_lt
                )
                nc.vector.tensor_tensor(
                    dst[:, cols], gt[:], lt[:], mybir.AluOpType.subtract
                )

        # Resident ternary weights (bf16): t1 = 8 x [128, hsh], t2 = 16 x [128, dout].
        t1_sb = [
            t1p.tile([P, hsh], FP8, tag=f"t1_{k}", name=f"t1_{k}")
            for k in range(nkt1)
        ]
        for kt in range(nkt1):
            ternarize_ktile(t1_sb[kt], w1c, n1c, kt, hsh)
        t2_sb = [
            t2p.tile([P, dout], FP8, tag=f"t2_{k}", name=f"t2_{k}")
            for k in range(nkt2)
        ]
        for kt in range(nkt2):
            ternarize_ktile(t2_sb[kt], w2c, n2c, kt, dout)

        for ch in range(nch):
            bcols = slice(ch * cb, (ch + 1) * cb)

            # Batch chunk of xT: [128, nkt1, cb] bf16.
            xb = xp.tile([P, nkt1, cb], BF16, tag="xb")
            nc.sync.dma_start(xb[:], xT3[:, :, bcols])

            # L1: hT[m] = tanh((t1.T @ xT)[m] * s1[m] + b1[m])  (bf16)
            h = hp.tile([P, nm1, cb], BF16, tag="h")
            for m in range(nm1):
                acc = ps1.tile([P, cb], F32, tag="ps1")
                for kt in range(nkt1):
                    nc.tensor.matmul(
                        acc[:],
                        t1_sb[kt][:, m * P : (m + 1) * P],
                        xb[:, kt, :],
                        start=(kt == 0),
                        stop=(kt == nkt1 - 1),
                    )
                nc.scalar.activation(
                    h[:, m, :],
                    acc[:],
                    mybir.ActivationFunctionType.Tanh,
                    bias=b1_sb[:, m : m + 1],
                    scale=s1_sb[:, m : m + 1],
                )

            # L2: poutT[mo] = (t2.T @ hT)[mo]  (fp32 partial, pre s2/b2)
            for mo in range(nm2):
                acc = ps2.tile([P, cb], F32, tag="ps2")
                for kt in range(nkt2):
                    nc.tensor.matmul(
                        acc[:],
                        t2_sb[kt][:, mo * P : (mo + 1) * P],
                        h[:, kt, :],
                        start=(kt == 0),
                        stop=(kt == nkt2 - 1),
                    )
                st = stp.tile([P, cb], F32, tag="st")
                nc.vector.tensor_copy(st[:], acc[:])
                g, mg = divmod(mo, mo_per_g)
                nc.sync.dma_start(
                    partials[ch][g][mg * P : (mg + 1) * P, :], st[:]
                )
                if mg == mo_per_g - 1:
                    # This 1024-row group is fully staged: reduce it now.
                    # Core c receives channels [g*1024 + c*128, +128).
                    nc.gpsimd.collective_compute(
                        "ReduceScatter",
                        mybir.AluOpType.add,
                        replica_groups=groups,
                        ins=[partials[ch][g][:]],
                        outs=[rs_outs[ch][g][:]],
                    )

            # Apply s2/b2 to the owned slabs and emit.
            for r in range(nor):
                rt = rsp.tile([P, cb], F32, tag="rs")
                nc.sync.dma_start(rt[:], rs_outs[ch][r][:])
                nc.vector.tensor_scalar(
                    rt[:],
                    rt[:],
                    s2_sb[:, r : r + 1],
                    b2_sb[:, r : r + 1],
                    mybir.AluOpType.mult,
                    mybir.AluOpType.add,
                )
                nc.sync.dma_start(outT[r * P : (r + 1) * P, bcols], rt[:])

    nc.compile()
    return nc


def TileCtx(nc):
    return tile.TileContext(nc)


def _chan_perm(c, w=W, dout=DOUT):
    """Output channels owned by core c, in shard-row order: for each RS
    group g (w*128 rows), core c gets rows [c*128, (c+1)*128)."""
    P = 128
    rs_rows = w * P
    nrs = dout // rs_rows
    return np.concatenate(
        [np.arange(g * rs_rows + c * P, g * rs_rows + (c + 1) * P) for g in range(nrs)]
    )


def _shard_inputs(x, w1, s1, b1, w2, s2, b2, n1, n2, w=W, dh=DH, dout=DOUT):
    P = 128
    hsh = dh // w
    osh = dout // w
    nm1 = hsh // P
    nor = osh // P
    xT = np.ascontiguousarray(x.T).astype(ml_dtypes.bfloat16)
    in_maps = []
    for c in range(w):
        hs = slice(c * hsh, (c + 1) * hsh)
        chans = _chan_perm(c, w=w, dout=dout)
        in_maps.append(
            {
                "xT": xT,
                "w1c": np.ascontiguousarray(w1[:, hs]),
                "n1c": np.ascontiguousarray(n1[:, hs]),
                "w2c": np.ascontiguousarray(w2[hs, :]),
                "n2c": np.ascontiguousarray(n2[hs, :]),
                "s1c": np.ascontiguousarray(s1[hs].reshape(nm1, P).T),
                "b1c": np.ascontiguousarray(b1[hs].reshape(nm1, P).T),
                "s2c": np.ascontiguousarray(s2[chans].reshape(nor, P).T),
                "b2c": np.ascontiguousarray(b2[chans].reshape(nor, P).T),
            }
        )
    return in_maps


_NC_CACHE = {}


def kernel(**inputs) -> np.ndarray:
    global LAST_RUN
    x = np.asarray(inputs["x"], dtype=np.float32)
    w1 = np.asarray(inputs["w1"], dtype=np.float32)
    s1 = np.asarray(inputs["s1"], dtype=np.float32)
    b1 = np.asarray(inputs["b1"], dtype=np.float32)
    w2 = np.asarray(inputs["w2"], dtype=np.float32)
    s2 = np.asarray(inputs["s2"], dtype=np.float32)
    b2 = np.asarray(inputs["b2"], dtype=np.float32)
    n1 = np.asarray(inputs["n1"], dtype=np.float32)
    n2 = np.asarray(inputs["n2"], dtype=np.float32)
    scale = float(np.asarray(inputs["scale"]))

    key = scale
    if key not in _NC_CACHE:
        _NC_CACHE[key] = build_decoder_nc(scale)
    nc = _NC_CACHE[key]

    in_maps = _shard_inputs(x, w1, s1, b1, w2, s2, b2, n1, n2)
    trace = bool(int(os.environ.get("KERNEL_TRACE", "0")))
    res = run_bass_kernel_spmd(
        nc, in_maps, core_ids=list(range(W)), trace=trace
    )
    LAST_RUN = res

    outT = np.empty((DOUT, B), np.float32)
    for c in range(W):
        outT[_chan_perm(c)] = np.asarray(res.results[c]["outT"])
    out = np.ascontiguousarray(outT.T).reshape(B, 3, 32, 32).astype(np.float32)
    return out


# revision 11
# speedup vs baseline: 1.2177x; 1.0118x over previous
"""Trainium2 Bass kernel for the ternary-MLP decoder.

  h   = tanh(x @ (s1 * tern(w1 - scale*n1)) + b1)
  out = (h @ (s2 * tern(w2 - scale*n2)) + b2).reshape(-1, 3, 32, 32)

Strategy (8 NeuronCores, Megatron tensor-parallel over D_H):
  - core c owns h-columns [c*2048, (c+1)*2048): w1/s1/b1 column shard,
    w2 row shard. Full batch on every core.
  - All matmuls computed in transposed space: hT = t1c.T @ xT,
    poutT = t2c.T @ hT, so the natural [K, M] layouts of w1/w2 feed the
    PE stationary operand directly and no on-device transposes happen.
    Host passes xT (bf16) and reassembles outT at the end.
  - Ternarization runs on-device in exact fp32 (bitwise-identical
    compares to the reference); ternary weights are cast to bf16 which
    is exact for {-1, 0, 1}. Matmuls run bf16 with fp32 PSUM accum.
  - The [3072, B] partial outputs are summed across cores with a
    chunked ReduceScatter (batch chunks of 512) that overlaps compute;
    core c ends up with outT rows [c*384, (c+1)*384) for all batch.
"""

import os
from contextlib import ExitStack

import ml_dtypes
import numpy as np

import concourse.bass as bass
import concourse.tile as tile
from concourse import bacc, mybir
from concourse.bass_utils import run_bass_kernel_spmd

F32 = mybir.dt.float32
BF16 = mybir.dt.bfloat16
FP8 = mybir.dt.float8e4

# Problem dims (hardcoded per contract).
B, DIN, DH, DOUT = 4096, 1024, 16384, 3 * 32 * 32
W = 8  # cores

# Results of the last traced run (for test harness inspection).
LAST_RUN = None


def build_decoder_nc(
    scale: float,
    b: int = B,
    din: int = DIN,
    dh: int = DH,
    dout: int = DOUT,
    w: int = W,
    cb: int = 512,
    tern_f: int = 512,
):
    """Build the per-core Bass program (same program for all cores; the
    per-core shards arrive as inputs)."""
    P = 128
    hsh = dh // w        # h columns owned by this core
    osh = dout // w      # outT rows owned after ReduceScatter
    nkt1 = din // P      # L1 contraction tiles
    nkt2 = hsh // P      # L2 contraction tiles (== L1 output m-tiles)
    nm1 = hsh // P       # L1 output tiles (hT rows / P)
    nm2 = dout // P      # L2 output tiles (outT rows / P)
    nor = osh // P       # post-RS row tiles
    nch = b // cb        # batch chunks
    assert din % P == 0 and hsh % P == 0 and dout % P == 0 and osh % P == 0
    assert b % cb == 0 and cb <= 512

    nc = bacc.Bacc(None, num_devices=w)

    xT = nc.dram_tensor("xT", [din, b], BF16, kind="ExternalInput")
    w1c = nc.dram_tensor("w1c", [din, hsh], F32, kind="ExternalInput")
    n1c = nc.dram_tensor("n1c", [din, hsh], F32, kind="ExternalInput")
    w2c = nc.dram_tensor("w2c", [hsh, dout], F32, kind="ExternalInput")
    n2c = nc.dram_tensor("n2c", [hsh, dout], F32, kind="ExternalInput")
    s1c = nc.dram_tensor("s1c", [P, nm1], F32, kind="ExternalInput")
    b1c = nc.dram_tensor("b1c", [P, nm1], F32, kind="ExternalInput")
    s2c = nc.dram_tensor("s2c", [P, nor], F32, kind="ExternalInput")
    b2c = nc.dram_tensor("b2c", [P, nor], F32, kind="ExternalInput")
    outT = nc.dram_tensor("outT", [osh, b], F32, kind="ExternalOutput")

    # Per-chunk DRAM buffers for the cross-core reduction. Each chunk's
    # [dout, cb] partial is reduced as `nrs` independent ReduceScatters of
    # w*P rows each, so every RS yields exactly one [P, cb] tile per core
    # and the HBM bursts stay small enough to hide behind compute.
    nrs = nor  # one RS group per post-RS row tile
    rs_rows = dout // nrs
    assert rs_rows % (w * P) == 0 and rs_rows // w == P
    partials = [
        [nc.dram_tensor(f"partial_{i}_{g}", [rs_rows, cb], F32) for g in range(nrs)]
        for i in range(nch)
    ]
    rs_outs = [
        [nc.dram_tensor(f"rs_out_{i}_{g}", [P, cb], F32) for g in range(nrs)]
        for i in range(nch)
    ]
    groups = [list(range(w))]
    mo_per_g = nm2 // nrs

    xT3 = xT.rearrange("(ko p) b -> p ko b", p=P)

    with TileCtx(nc) as tc, ExitStack() as ctx:
        consts = ctx.enter_context(tc.tile_pool(name="consts", bufs=1))
        t1p = ctx.enter_context(tc.tile_pool(name="t1", bufs=1))
        t2p = ctx.enter_context(tc.tile_pool(name="t2", bufs=1))
        wnp = ctx.enter_context(tc.tile_pool(name="wn", bufs=3))
        gtp = ctx.enter_context(tc.tile_pool(name="gt", bufs=2))
        xp = ctx.enter_context(tc.tile_pool(name="xb", bufs=4))
        hp = ctx.enter_context(tc.tile_pool(name="h", bufs=4))
        stp = ctx.enter_context(tc.tile_pool(name="stage", bufs=8))
        rsp = ctx.enter_context(tc.tile_pool(name="rsp", bufs=4))
        ps1 = ctx.enter_context(tc.tile_pool(name="ps1", bufs=2, space="PSUM"))
        ps2 = ctx.enter_context(tc.tile_pool(name="ps2", bufs=6, space="PSUM"))

        # Per-partition scale/bias vectors (host pre-arranged as [128, m]).
        s1_sb = consts.tile([P, nm1], F32, tag="s1")
        b1_sb = consts.tile([P, nm1], F32, tag="b1")
        s2_sb = consts.tile([P, nor], F32, tag="s2")
        b2_sb = consts.tile([P, nor], F32, tag="b2")
        nc.sync.dma_start(s1_sb[:], s1c[:])
        nc.sync.dma_start(b1_sb[:], b1c[:])
        nc.sync.dma_start(s2_sb[:], s2c[:])
        nc.sync.dma_start(b2_sb[:], b2c[:])

        def ternarize_ktile(dst, w_dram, n_dram, kt, fdim):
            """dst[:, 0:fdim] (bf16) = (q > 1) - (q < -1), q = w - scale*n.

            Exact fp32 compares, identical to the reference semantics."""
            fw = min(tern_f, fdim)
            assert fdim % fw == 0
            for j in range(fdim // fw):
                cols = slice(j * fw, (j + 1) * fw)
                wt = wnp.tile([P, fw], F32, tag="w")
                nc.sync.dma_start(wt[:], w_dram[kt * P : (kt + 1) * P, cols])
                if scale != 0.0:
                    nt = wnp.tile([P, fw], F32, tag="n")
                    nc.sync.dma_start(nt[:], n_dram[kt * P : (kt + 1) * P, cols])
                    if scale != 1.0:
                        nc.vector.tensor_scalar(
                            nt[:], nt[:], float(scale), None, mybir.AluOpType.mult
                        )
                    nc.vector.tensor_tensor(
                        wt[:], wt[:], nt[:], mybir.AluOpType.subtract
                    )
                gt = gtp.tile([P, fw], BF16, tag="gt")
                nc.vector.tensor_scalar(
                    gt[:], wt[:], 1.0, None, mybir.AluOpType.is_gt
                )
                lt = gtp.tile([P, fw], BF16, tag="lt")
                nc.vector.tensor_scalar(
                    lt[:], wt[:], -1.0, None, mybir.AluOpType.is_lt
                )
                nc.vector.tensor_tensor(
                    dst[:, cols], gt[:], lt[:], mybir.AluOpType.subtract
                )

        # Resident ternary weights (bf16): t1 = 8 x [128, hsh], t2 = 16 x [128, dout].
        t1_sb = [
            t1p.tile([P, hsh], FP8, tag=f"t1_{k}", name=f"t1_{k}")
            for k in range(nkt1)
        ]
        for kt in range(nkt1):
            ternarize_ktile(t1_sb[kt], w1c, n1c, kt, hsh)
        # Prefetch the first few batch chunks of xT BEFORE the (large) t2
        # weight streams enter the DMA queues, so L1 of chunks 0..3 has its
        # inputs while t2 is still being ternarized.
        xb_tiles = {}
        n_prefetch = min(4, nch)
        for ch in range(n_prefetch):
            xb = xp.tile([P, nkt1, cb], BF16, tag="xb", name=f"xb_{ch}")
            nc.sync.dma_start(xb[:], xT3[:, :, ch * cb : (ch + 1) * cb])
            xb_tiles[ch] = xb

        t2_sb = [
            t2p.tile([P, dout], FP8, tag=f"t2_{k}", name=f"t2_{k}")
            for k in range(nkt2)
        ]
        for kt in range(nkt2):
            ternarize_ktile(t2_sb[kt], w2c, n2c, kt, dout)

        for ch in range(nch):
            bcols = slice(ch * cb, (ch + 1) * cb)

            # Batch chunk of xT: [128, nkt1, cb] bf16.
            if ch in xb_tiles:
                xb = xb_tiles[ch]
            else:
                xb = xp.tile([P, nkt1, cb], BF16, tag="xb", name=f"xb_{ch}")
                nc.sync.dma_start(xb[:], xT3[:, :, bcols])

            # L1: hT[m] = tanh((t1.T @ xT)[m] * s1[m] + b1[m])  (bf16)
            h = hp.tile([P, nm1, cb], BF16, tag="h")
            for m in range(nm1):
                acc = ps1.tile([P, cb], F32, tag="ps1")
                for kt in range(nkt1):
                    nc.tensor.matmul(
                        acc[:],
                        t1_sb[kt][:, m * P : (m + 1) * P],
                        xb[:, kt, :],
                        start=(kt == 0),
                        stop=(kt == nkt1 - 1),
                    )
                nc.scalar.activation(
                    h[:, m, :],
                    acc[:],
                    mybir.ActivationFunctionType.Tanh,
                    bias=b1_sb[:, m : m + 1],
                    scale=s1_sb[:, m : m + 1],
                )

            # L2: poutT[mo] = (t2.T @ hT)[mo]  (fp32 partial, pre s2/b2)
            for mo in range(nm2):
                acc = ps2.tile([P, cb], F32, tag="ps2")
                for kt in range(nkt2):
                    nc.tensor.matmul(
                        acc[:],
                        t2_sb[kt][:, mo * P : (mo + 1) * P],
                        h[:, kt, :],
                        start=(kt == 0),
                        stop=(kt == nkt2 - 1),
                    )
                st = stp.tile([P, cb], F32, tag="st")
                nc.vector.tensor_copy(st[:], acc[:])
                g, mg = divmod(mo, mo_per_g)
                nc.sync.dma_start(
                    partials[ch][g][mg * P : (mg + 1) * P, :], st[:]
                )
                if mg == mo_per_g - 1:
                    # This 1024-row group is fully staged: reduce it now.
                    # Core c receives channels [g*1024 + c*128, +128).
                    nc.gpsimd.collective_compute(
                        "ReduceScatter",
                        mybir.AluOpType.add,
                        replica_groups=groups,
                        ins=[partials[ch][g][:]],
                        outs=[rs_outs[ch][g][:]],
                    )

            # Apply s2/b2 to the owned slabs and emit.
            for r in range(nor):
                rt = rsp.tile([P, cb], F32, tag="rs")
                nc.sync.dma_start(rt[:], rs_outs[ch][r][:])
                nc.vector.tensor_scalar(
                    rt[:],
                    rt[:],
                    s2_sb[:, r : r + 1],
                    b2_sb[:, r : r + 1],
                    mybir.AluOpType.mult,
                    mybir.AluOpType.add,
                )
                nc.sync.dma_start(outT[r * P : (r + 1) * P, bcols], rt[:])

    nc.compile()
    return nc


def TileCtx(nc):
    return tile.TileContext(nc)


def _chan_perm(c, w=W, dout=DOUT):
    """Output channels owned by core c, in shard-row order: for each RS
    group g (w*128 rows), core c gets rows [c*128, (c+1)*128)."""
    P = 128
    rs_rows = w * P
    nrs = dout // rs_rows
    return np.concatenate(
        [np.arange(g * rs_rows + c * P, g * rs_rows + (c + 1) * P) for g in range(nrs)]
    )


def _shard_inputs(x, w1, s1, b1, w2, s2, b2, n1, n2, w=W, dh=DH, dout=DOUT):
    P = 128
    hsh = dh // w
    osh = dout // w
    nm1 = hsh // P
    nor = osh // P
    xT = np.ascontiguousarray(x.T).astype(ml_dtypes.bfloat16)
    in_maps = []
    for c in range(w):
        hs = slice(c * hsh, (c + 1) * hsh)
        chans = _chan_perm(c, w=w, dout=dout)
        in_maps.append(
            {
                "xT": xT,
                "w1c": np.ascontiguousarray(w1[:, hs]),
                "n1c": np.ascontiguousarray(n1[:, hs]),
                "w2c": np.ascontiguousarray(w2[hs, :]),
                "n2c": np.ascontiguousarray(n2[hs, :]),
                "s1c": np.ascontiguousarray(s1[hs].reshape(nm1, P).T),
                "b1c": np.ascontiguousarray(b1[hs].reshape(nm1, P).T),
                "s2c": np.ascontiguousarray(s2[chans].reshape(nor, P).T),
                "b2c": np.ascontiguousarray(b2[chans].reshape(nor, P).T),
            }
        )
    return in_maps


_NC_CACHE = {}


def kernel(**inputs) -> np.ndarray:
    global LAST_RUN
    x = np.asarray(inputs["x"], dtype=np.float32)
    w1 = np.asarray(inputs["w1"], dtype=np.float32)
    s1 = np.asarray(inputs["s1"], dtype=np.float32)
    b1 = np.asarray(inputs["b1"], dtype=np.float32)
    w2 = np.asarray(inputs["w2"], dtype=np.float32)
    s2 = np.asarray(inputs["s2"], dtype=np.float32)
    b2 = np.asarray(inputs["b2"], dtype=np.float32)
    n1 = np.asarray(inputs["n1"], dtype=np.float32)
    n2 = np.asarray(inputs["n2"], dtype=np.float32)
    scale = float(np.asarray(inputs["scale"]))

    key = scale
    if key not in _NC_CACHE:
        _NC_CACHE[key] = build_decoder_nc(scale)
    nc = _NC_CACHE[key]

    in_maps = _shard_inputs(x, w1, s1, b1, w2, s2, b2, n1, n2)
    trace = bool(int(os.environ.get("KERNEL_TRACE", "0")))
    res = run_bass_kernel_spmd(
        nc, in_maps, core_ids=list(range(W)), trace=trace
    )
    LAST_RUN = res

    outT = np.empty((DOUT, B), np.float32)
    for c in range(W):
        outT[_chan_perm(c)] = np.asarray(res.results[c]["outT"])
    out = np.ascontiguousarray(outT.T).reshape(B, 3, 32, 32).astype(np.float32)
    return out
